# revision 5
# baseline (speedup 1.0000x reference)
"""RBF-kernel multi-head attention on 8 TRN2 NeuronCores, data-parallel over batch.

Per core (one batch element b):
  qh/kh/vh = x @ W   (heads), computed in transposed [head_dim, T] layout
  l2[t,t'] = |q_t|^2 + |k_t'|^2 - 2 q_t.k_t'   via one K=66 augmented matmul
  attn = exp(-l2)  (computed twice: natural layout for the HBM output,
                    transposed layout to feed attn @ v)
  out = (attn @ vh) @ Wo

Precision: projections + scores in fp32r (~17-bit mantissa, verified 1.3e-4
rel on HW), attn@v and output projection in bf16 (only affects `out`).
"""
from contextlib import ExitStack

import numpy as np

import concourse.bass as bass
import concourse.mybir as mybir
import concourse.tile as tile
from concourse import bacc, bass_utils
from concourse.masks import make_identity

F32 = mybir.dt.float32
F32R = mybir.dt.float32r
BF16 = mybir.dt.bfloat16
EXP = mybir.ActivationFunctionType.Exp

B, T, D, H, KD = 8, 1024, 512, 8, 64
NT = T // 128   # 8   t-tiles (partition tiles)
KT = D // 128   # 4   d-tiles (contraction tiles)
NN = T // 512   # 2   moving-free tiles of 512


def _body(ctx: ExitStack, nc, tc, q_in, k_in, v_in, Wq, Wk, Wv, Wo, out_t, attn_t):
    singles = ctx.enter_context(tc.tile_pool(name="singles", bufs=1))
    tmp = ctx.enter_context(tc.tile_pool(name="tmp", bufs=3))
    wtmp = ctx.enter_context(tc.tile_pool(name="wtmp", bufs=2))
    sqp = ctx.enter_context(tc.tile_pool(name="sqp", bufs=2))
    stp = ctx.enter_context(tc.tile_pool(name="stp", bufs=3))
    snp = ctx.enter_context(tc.tile_pool(name="snp", bufs=3))
    otp = ctx.enter_context(tc.tile_pool(name="otp", bufs=2))
    ps = ctx.enter_context(tc.tile_pool(name="ps", bufs=3, space="PSUM"))
    ps2 = ctx.enter_context(tc.tile_pool(name="ps2", bufs=3, space="PSUM"))
    psat = ctx.enter_context(tc.tile_pool(name="psat", bufs=1, space="PSUM"))

    # --- constants ---------------------------------------------------------
    ident = singles.tile([128, 128], F32, tag="ident")
    make_identity(nc, ident)
    mask_q = singles.tile([64, 2], F32, tag="mask_q")   # -> psum rows [q2, 0]
    nc.vector.memset(mask_q[:, 0:1], 0.25)              # 0.25: q rows carry -2q
    nc.vector.memset(mask_q[:, 1:2], 0.0)
    mask_k = singles.tile([64, 2], F32, tag="mask_k")   # -> psum rows [0, k2]
    nc.vector.memset(mask_k[:, 0:1], 0.0)
    nc.vector.memset(mask_k[:, 1:2], 1.0)
    # per-partition selectors for rows 64/65: s1 = [1, 0], s2 = [0, 1]
    sel1 = singles.tile([128, 1], F32, tag="sel1")
    nc.vector.memset(sel1[64:66, :], 0.0)
    nc.vector.memset(sel1[64:65, :], 1.0)
    sel2 = singles.tile([128, 1], F32, tag="sel2")
    nc.vector.tensor_scalar(out=sel2[64:66, :], in0=sel1[64:66, :],
                            scalar1=-1.0, scalar2=1.0,
                            op0=mybir.AluOpType.mult, op1=mybir.AluOpType.add)

    # --- weights: load fp32, cast (and fold -2 into Wq) --------------------
    w_r = {}
    for name, w_in, scale, dt in (("wq", Wq, -2.0, F32R), ("wk", Wk, 1.0, F32R),
                                  ("wv", Wv, 1.0, F32R), ("wo", Wo, 1.0, BF16)):
        wt = wtmp.tile([128, KT, 512], F32, tag="wtmp")
        nc.sync.dma_start(out=wt, in_=w_in.ap().rearrange("(kt p) n -> p kt n", p=128))
        wr = singles.tile([128, KT, 512], dt, tag=name)
        if scale == 1.0:
            nc.vector.tensor_copy(wr, wt)
        else:
            nc.vector.tensor_scalar_mul(wr, wt, scale)
        w_r[name] = wr
    wq_r, wk_r, wv_r, wo_b = w_r["wq"], w_r["wk"], w_r["wv"], w_r["wo"]

    # --- load inputs natural, transpose to [d, t] via PE -------------------
    xT = {}
    for name, x_in in (("q", q_in), ("k", k_in), ("v", v_in)):
        xt = singles.tile([128, KT, T], F32R, tag=f"{name}T")
        xT[name] = xt
        for nt in range(NT):
            xnat = tmp.tile([128, 512], F32, tag="nat")
            nc.sync.dma_start(out=xnat, in_=x_in.ap()[nt * 128:(nt + 1) * 128, :])
            for kt in range(KT):
                pt = ps.tile([128, 512], F32, tag="ps")
                nc.tensor.transpose(pt[:, 0:128], xnat[:, kt * 128:(kt + 1) * 128],
                                    ident)
                nc.vector.tensor_copy(xt[:, kt, nt * 128:(nt + 1) * 128], pt[:, 0:128])
    qT, kT, vT = xT["q"], xT["k"], xT["v"]

    # --- q/k projections into augmented [66, H, T] tensors -----------------
    # qa rows: 0-63 = -2*qh^T, 64 = |q|^2, 65 = 1
    # ka rows: 0-63 =    kh^T, 64 = 1,     65 = |k|^2
    qa = singles.tile([66, H, T], F32R, tag="qa")
    ka = singles.tile([66, H, T], F32R, tag="ka")
    for name, aug, wr, xt in (("q", qa, wq_r, qT), ("k", ka, wk_r, kT)):
        for h in range(H):
            for n in range(NN):
                pp = ps.tile([128, 512], F32, tag="ps")
                for kt in range(KT):
                    nc.tensor.matmul(pp[0:64, :], wr[:, kt, h * 64:(h + 1) * 64],
                                     xt[:, kt, n * 512:(n + 1) * 512],
                                     start=(kt == 0), stop=(kt == KT - 1))
                nsl = slice(n * 512, (n + 1) * 512)
                nc.vector.tensor_copy(aug[0:64, h, nsl], pp[0:64, :])
                sq = sqp.tile([64, 512], F32, tag="sq")
                nc.scalar.activation(out=sq, in_=pp[0:64, :],
                                     func=mybir.ActivationFunctionType.Square)
                pq = ps.tile([128, 512], F32, tag="ps")
                if name == "q":
                    # psum rows 64:66 = [q2, 0] -> aug rows [q2, 1]
                    nc.tensor.matmul(pq[64:66, :], mask_q, sq, start=True, stop=True)
                    s_mul, s_add = sel1, sel2
                else:
                    # psum rows 64:66 = [0, k2] -> aug rows [1, k2]
                    nc.tensor.matmul(pq[64:66, :], mask_k, sq, start=True, stop=True)
                    s_mul, s_add = sel2, sel1
                nc.vector.tensor_scalar(out=aug[64:66, h, nsl], in0=pq[64:66, :],
                                        scalar1=s_mul[64:66, :],
                                        scalar2=s_add[64:66, :],
                                        op0=mybir.AluOpType.mult,
                                        op1=mybir.AluOpType.add)

    # --- v projection (natural layout, bf16, 2 heads per matmul) -----------
    vh = singles.tile([128, NT, 512], BF16, tag="vh")
    for tt in range(NT):
        pv = ps.tile([128, 512], F32, tag="ps")
        for kt in range(KT):
            nc.tensor.matmul(pv, vT[:, kt, tt * 128:(tt + 1) * 128], wv_r[:, kt, :],
                             start=(kt == 0), stop=(kt == KT - 1))
        nc.vector.tensor_copy(vh[:, tt, :], pv)

    # --- scores, exp, attn@v, attn output ----------------------------------
    aT = singles.tile([128, KT, T], BF16, tag="aT")   # (attn @ vh)^T, hd-major
    pat = None
    for h in range(H):
        if h % 2 == 0:
            pat = psat.tile([128, T], F32, tag="at")
        hb = (h % 2) * 64
        # transposed scores S^T[t', t] -> exp -> bf16 -> accumulate A^T
        for m in range(NT):
            st = stp.tile([128, T], BF16, tag="st")
            for n in range(NN):
                pst = ps2.tile([128, 512], F32, tag="ps2")
                nc.tensor.matmul(pst, ka[:, h, m * 128:(m + 1) * 128],
                                 qa[:, h, n * 512:(n + 1) * 512],
                                 start=True, stop=True)
                nc.scalar.activation(out=st[:, n * 512:(n + 1) * 512], in_=pst,
                                     func=EXP, scale=-1.0)
            for n in range(NN):
                nc.tensor.matmul(pat[hb:hb + 64, n * 512:(n + 1) * 512],
                                 vh[:, m, h * 64:(h + 1) * 64],
                                 st[:, n * 512:(n + 1) * 512],
                                 start=(m == 0), stop=(m == NT - 1))
        if h % 2 == 1:
            nc.vector.tensor_copy(aT[:, h // 2, :], pat)
        # natural scores S[t, t'] -> exp -> HBM
        for m in range(NT):
            sn = snp.tile([128, T], F32, tag="sn")
            for n in range(NN):
                psn = ps2.tile([128, 512], F32, tag="ps2")
                nc.tensor.matmul(psn, qa[:, h, m * 128:(m + 1) * 128],
                                 ka[:, h, n * 512:(n + 1) * 512],
                                 start=True, stop=True)
                nc.scalar.activation(out=sn[:, n * 512:(n + 1) * 512], in_=psn,
                                     func=EXP, scale=-1.0)
            nc.sync.dma_start(out=attn_t.ap()[h, m * 128:(m + 1) * 128, :], in_=sn)

    # --- output projection: out = A @ Wo (bf16) ----------------------------
    for tt in range(NT):
        po = ps.tile([128, 512], F32, tag="ps")
        for g in range(KT):
            nc.tensor.matmul(po, aT[:, g, tt * 128:(tt + 1) * 128], wo_b[:, g, :],
                             start=(g == 0), stop=(g == KT - 1))
        ot = otp.tile([128, 512], F32, tag="ot")
        nc.vector.tensor_copy(ot, po)
        nc.sync.dma_start(out=out_t.ap()[tt * 128:(tt + 1) * 128, :], in_=ot)


def _build():
    nc = bacc.Bacc("TRN2", target_bir_lowering=False, debug=False)
    q_in = nc.dram_tensor("query", [T, D], F32, kind="ExternalInput")
    k_in = nc.dram_tensor("key", [T, D], F32, kind="ExternalInput")
    v_in = nc.dram_tensor("value", [T, D], F32, kind="ExternalInput")
    Wq = nc.dram_tensor("Wq", [D, H * KD], F32, kind="ExternalInput")
    Wk = nc.dram_tensor("Wk", [D, H * KD], F32, kind="ExternalInput")
    Wv = nc.dram_tensor("Wv", [D, H * KD], F32, kind="ExternalInput")
    Wo = nc.dram_tensor("Wo", [H * KD, D], F32, kind="ExternalInput")
    out_t = nc.dram_tensor("out", [T, D], F32, kind="ExternalOutput")
    attn_t = nc.dram_tensor("attn", [H, T, T], F32, kind="ExternalOutput")

    with tile.TileContext(nc) as tc:
        with ExitStack() as ctx:
            _body(ctx, nc, tc, q_in, k_in, v_in, Wq, Wk, Wv, Wo, out_t, attn_t)
    nc.compile()
    return nc


_NC = None


def _get_nc():
    global _NC
    if _NC is None:
        _NC = _build()
    return _NC


def run_sharded(in_maps, trace=False):
    nc = _get_nc()
    return bass_utils.run_bass_kernel_spmd(nc, in_maps, core_ids=list(range(B)),
                                           trace=trace)


def make_in_maps(query, key, value, Wq, Wk, Wv, Wo):
    query, key, value = (np.asarray(x, dtype=np.float32) for x in (query, key, value))
    Wq, Wk, Wv, Wo = (np.asarray(x, dtype=np.float32) for x in (Wq, Wk, Wv, Wo))
    return [
        {"query": np.ascontiguousarray(query[b]),
         "key": np.ascontiguousarray(key[b]),
         "value": np.ascontiguousarray(value[b]),
         "Wq": Wq, "Wk": Wk, "Wv": Wv, "Wo": Wo}
        for b in range(B)
    ]


def kernel(query, key, value, Wq, Wk, Wv, Wo):
    in_maps = make_in_maps(query, key, value, Wq, Wk, Wv, Wo)
    res = run_sharded(in_maps)
    out = np.stack([res.results[b]["out"] for b in range(B)])
    attn = np.stack([res.results[b]["attn"] for b in range(B)])
    return out, attn


# revision 29
# speedup vs baseline: 1.5296x; 1.5296x over previous
"""RBF-kernel multi-head attention on 8 TRN2 NeuronCores, data-parallel over batch.

Per core (one batch element b):
  qh/kh/vh = x @ W   (heads), computed in transposed [head_dim, T] layout
  l2[t,t'] = |q_t|^2 + |k_t'|^2 - 2 q_t.k_t'   via one K=66 augmented matmul
  attn = exp(-l2)  (computed twice: natural layout for the HBM output,
                    transposed layout to feed attn @ v)
  out = (attn @ vh) @ Wo

Precision: projections + scores in fp32r (~17-bit mantissa, verified 1.3e-4
rel on HW), attn@v and output projection in bf16 (only affects `out`).

Loop structure interleaves per-head projection with scores so the scalar
engine (exp) starts ~25us in instead of waiting for all projections.
"""
from contextlib import ExitStack

import numpy as np

import concourse.bass as bass
import concourse.mybir as mybir
import concourse.tile as tile
from concourse import bacc, bass_utils
from concourse.masks import make_identity

F32 = mybir.dt.float32
F32R = mybir.dt.float32r
BF16 = mybir.dt.bfloat16
EXP = mybir.ActivationFunctionType.Exp
SQUARE = mybir.ActivationFunctionType.Square
MULT = mybir.AluOpType.mult
ADD = mybir.AluOpType.add

B, T, D, H, KD = 8, 1024, 512, 8, 64
NT = T // 128   # 8   t-tiles (partition tiles)
KT = D // 128   # 4   d-tiles (contraction tiles)
NN = T // 512   # 2   moving-free tiles of 512


def _body(ctx: ExitStack, nc, tc, q_in, k_in, v_in, Wq, Wk, Wv, Wo, out_t, attn_t):
    singles = ctx.enter_context(tc.tile_pool(name="singles", bufs=1))
    tmp = ctx.enter_context(tc.tile_pool(name="tmp", bufs=2))
    wtmp = ctx.enter_context(tc.tile_pool(name="wtmp", bufs=1))
    sqp = ctx.enter_context(tc.tile_pool(name="sqp", bufs=2))
    stp = ctx.enter_context(tc.tile_pool(name="stp", bufs=4))
    snp = ctx.enter_context(tc.tile_pool(name="snp", bufs=4))
    otp = ctx.enter_context(tc.tile_pool(name="otp", bufs=2))
    ps = ctx.enter_context(tc.tile_pool(name="ps", bufs=2, space="PSUM"))
    ps2 = ctx.enter_context(tc.tile_pool(name="ps2", bufs=2, space="PSUM"))
    psat = ctx.enter_context(tc.tile_pool(name="psat", bufs=1, space="PSUM"))

    # --- constants ---------------------------------------------------------
    ident = singles.tile([128, 128], F32, tag="ident")
    make_identity(nc, ident)
    mask_q = singles.tile([64, 2], F32, tag="mask_q")   # -> psum rows [q2, 0]
    nc.vector.memset(mask_q[:, 0:1], 0.25)              # 0.25: q rows carry -2q
    nc.vector.memset(mask_q[:, 1:2], 0.0)
    mask_k = singles.tile([64, 2], F32, tag="mask_k")   # -> psum rows [0, k2]
    nc.vector.memset(mask_k[:, 0:1], 0.0)
    nc.vector.memset(mask_k[:, 1:2], 1.0)
    # per-partition selectors for rows 64/65: sel1 = [1, 0], sel2 = [0, 1]
    sel1 = singles.tile([128, 1], F32, tag="sel1")
    nc.vector.memset(sel1[64:66, :], 0.0)
    nc.vector.memset(sel1[64:65, :], 1.0)
    sel2 = singles.tile([128, 1], F32, tag="sel2")
    nc.vector.tensor_scalar(out=sel2[64:66, :], in0=sel1[64:66, :],
                            scalar1=-1.0, scalar2=1.0, op0=MULT, op1=ADD)

    # --- helpers: input transpose + weight load ----------------------------
    def load_transposed(x_in, xt, half=None):
        # 512KB DMAs (two 128-row tiles each) to stay off the descriptor floor
        rng = range(0, NT, 2) if half is None else \
            (range(0, NT // 2, 2) if half == 0 else range(NT // 2, NT, 2))
        for nt in rng:
            xnat = tmp.tile([128, 2, 512], F32, tag="nat")
            nc.sync.dma_start(
                out=xnat,
                in_=x_in.ap()[nt * 128:(nt + 2) * 128, :]
                    .rearrange("(a p) d -> p a d", p=128))
            for a in range(2):
                for kt in range(KT):
                    pt = ps.tile([128, 512], F32, tag="ps")
                    nc.tensor.transpose(pt[:, 0:128],
                                        xnat[:, a, kt * 128:(kt + 1) * 128], ident)
                    nc.vector.tensor_copy(
                        xt[:, kt, (nt + a) * 128:(nt + a + 1) * 128], pt[:, 0:128])

    def load_weight(w_in, scale, dt, name):
        wt = wtmp.tile([128, KT, 512], F32, tag="wtmp")
        nc.sync.dma_start(out=wt, in_=w_in.ap().rearrange("(kt p) n -> p kt n", p=128))
        wr = singles.tile([128, KT, 512], dt, tag=name)
        # gpsimd: keep the weight casts off the DVE queue during lead-in
        if scale == 1.0:
            nc.gpsimd.tensor_copy(wr, wt)
        else:
            nc.gpsimd.tensor_scalar_mul(wr, wt, scale)
        return wr

    # lead-in: q/k inputs + transposes first so PE starts ASAP, then q/k
    # weights (needed by head-0 projection); v and Wv/Wo come later, hidden
    # behind head-0 scores.
    qT = singles.tile([128, KT, T], F32R, tag="qT")
    kT = singles.tile([128, KT, T], F32R, tag="kT")
    vT = singles.tile([128, KT, T], F32R, tag="vT")
    load_transposed(q_in, qT, half=0)       # q tiles 0-3: unblocks q-proj n=0
    wq_r = load_weight(Wq, -2.0, F32R, "wq")
    wk_r = load_weight(Wk, 1.0, F32R, "wk")
    load_transposed(k_in, kT)               # all of k: needed by first exp
    load_transposed(q_in, qT, half=1)

    # --- augmented projections, one head at a time -------------------------
    # qa rows: 0-63 = -2*qh^T, 64 = |q|^2, 65 = 1
    # ka rows: 0-63 =    kh^T, 64 = 1,     65 = |k|^2
    qa = singles.tile([66, H, T], F32R, tag="qa")
    ka = singles.tile([66, H, T], F32R, tag="ka")

    def proj_unit(name, h, n):
        aug, wr, xt = (qa, wq_r, qT) if name == "q" else (ka, wk_r, kT)
        if True:
            if True:
                pp = ps.tile([128, 512], F32, tag="ps")
                for kt in range(KT):
                    nc.tensor.matmul(pp[0:64, :], wr[:, kt, h * 64:(h + 1) * 64],
                                     xt[:, kt, n * 512:(n + 1) * 512],
                                     start=(kt == 0), stop=(kt == KT - 1))
                nsl = slice(n * 512, (n + 1) * 512)
                nc.vector.tensor_copy(aug[0:64, h, nsl], pp[0:64, :])
                # square the f32r-rounded values (bit-layout is fp32-compatible)
                sq = sqp.tile([64, 512], F32, tag="sq")
                rnd = aug[0:64, h, nsl].bitcast(F32)
                nc.vector.tensor_mul(sq, rnd, rnd)
                if name == "q":
                    # psum rows 64:66 = [q2, 0] -> aug rows [q2, 1]
                    nc.tensor.matmul(pp[64:66, :], mask_q, sq, start=True, stop=True)
                    s_mul, s_add = sel1, sel2
                else:
                    # psum rows 64:66 = [0, k2] -> aug rows [1, k2]
                    nc.tensor.matmul(pp[64:66, :], mask_k, sq, start=True, stop=True)
                    s_mul, s_add = sel2, sel1
                nc.vector.tensor_scalar(out=aug[64:66, h, nsl],
                                        in0=pp[64:66, :],
                                        scalar1=s_mul[64:66, :],
                                        scalar2=s_add[64:66, :],
                                        op0=MULT, op1=ADD)

    # --- v projection (natural layout, bf16, 2 heads per matmul) -----------
    vh = singles.tile([128, NT, 512], BF16, tag="vh")

    def vproj_unit(wv_r, tt):
        pv = ps.tile([128, 512], F32, tag="ps")
        for kt in range(KT):
            nc.tensor.matmul(pv, vT[:, kt, tt * 128:(tt + 1) * 128],
                             wv_r[:, kt, :], start=(kt == 0), stop=(kt == KT - 1))
        nc.vector.tensor_copy(vh[:, tt, :], pv)

    def vload_unit(nt):
        xnat = tmp.tile([128, 2, 512], F32, tag="nat")
        nc.sync.dma_start(
            out=xnat,
            in_=v_in.ap()[nt * 128:(nt + 2) * 128, :]
                .rearrange("(a p) d -> p a d", p=128))
        for a in range(2):
            for kt in range(KT):
                pt = ps.tile([128, 512], F32, tag="ps")
                nc.tensor.transpose(pt[:, 0:128],
                                    xnat[:, a, kt * 128:(kt + 1) * 128], ident)
                nc.vector.tensor_copy(
                    vT[:, kt, (nt + a) * 128:(nt + a + 1) * 128], pt[:, 0:128])

    # --- scores ------------------------------------------------------------
    aT = singles.tile([128, KT, T], BF16, tag="aT")   # (attn @ vh)^T, hd-major

    def sn_unit(h, m):
        # natural scores S[t, t'] -> exp -> HBM
        psn = ps2.tile([128, T], F32, tag="ps2")
        for n in range(NN):
            nc.tensor.matmul(psn[:, n * 512:(n + 1) * 512],
                             qa[:, h, m * 128:(m + 1) * 128],
                             ka[:, h, n * 512:(n + 1) * 512],
                             start=True, stop=True)
        sn = snp.tile([128, T], F32, tag="sn")
        nc.scalar.activation(out=sn, in_=psn, func=EXP, scale=-1.0)
        nc.sync.dma_start(out=attn_t.ap()[h, m * 128:(m + 1) * 128, :], in_=sn)

    def sT_unit(h, m, pat):
        # transposed scores S^T[t', t] -> exp -> bf16 -> accumulate A^T
        hb = (h % 2) * 64
        pst = ps2.tile([128, T], F32, tag="ps2")
        for n in range(NN):
            nc.tensor.matmul(pst[:, n * 512:(n + 1) * 512],
                             ka[:, h, m * 128:(m + 1) * 128],
                             qa[:, h, n * 512:(n + 1) * 512],
                             start=True, stop=True)
        st = stp.tile([128, T], BF16, tag="st")
        nc.scalar.activation(out=st, in_=pst, func=EXP, scale=-1.0)
        for n in range(NN):
            nc.tensor.matmul(pat[hb:hb + 64, n * 512:(n + 1) * 512],
                             vh[:, m, h * 64:(h + 1) * 64],
                             st[:, n * 512:(n + 1) * 512],
                             start=(m == 0), stop=(m == NT - 1))

    def outproj_unit(wo_b, tt):
        po = ps.tile([128, 512], F32, tag="ps")
        for g in range(KT):
            nc.tensor.matmul(po, aT[:, g, tt * 128:(tt + 1) * 128], wo_b[:, g, :],
                             start=(g == 0), stop=(g == KT - 1))
        ot = otp.tile([128, 512], F32, tag="ot")
        nc.vector.tensor_copy(ot, po)
        nc.sync.dma_start(out=out_t.ap()[tt * 128:(tt + 1) * 128, :], in_=ot)

    # --- main schedule -----------------------------------------------------
    # Per head, the 16 score tiles are the ACT-rate-limited stream; PE-side
    # work for the NEXT head (projection chunks) is injected between score
    # tiles so neither engine sees a long run of foreign work.
    for nm, n in (("q", 0), ("k", 0), ("k", 1), ("q", 1)):
        proj_unit(nm, 0, n)

    # head 0: natural scores first (ACT starts early), v pipeline interleaved
    pat = psat.tile([128, T], F32, tag="at")
    for m in range(NT):
        sn_unit(0, m)
        if m % 2 == 0:
            vload_unit(m)
        if m == 0:
            wv_r = load_weight(Wv, 1.0, F32R, "wv")
            wo_b = load_weight(Wo, 1.0, BF16, "wo")
    proj_next = [("q", 1, 0), ("q", 1, 1), ("k", 1, 0), ("k", 1, 1)]
    for m in range(NT):
        vproj_unit(wv_r, m)
        sT_unit(0, m, pat)
        if m % 2 == 1:
            nm, hh, n = proj_next[m // 2]
            proj_unit(nm, hh, n)

    # heads 1..6: sT stream, then sn stream with next head's projection mixed
    for h in range(1, H - 1):
        if h % 2 == 0:
            pat = psat.tile([128, T], F32, tag="at")
        for m in range(NT):
            sT_unit(h, m, pat)
        if h % 2 == 1:
            nc.vector.tensor_copy(aT[:, h // 2, :], pat)
        proj_next = [("q", h + 1, 0), ("q", h + 1, 1), ("k", h + 1, 0),
                     ("k", h + 1, 1)]
        for m in range(NT):
            sn_unit(h, m)
            if m % 2 == 1:
                nm, hh, n = proj_next[m // 2]
                proj_unit(nm, hh, n)

    # head 7: finish A^T first, then overlap outproj with the last sn stream
    h = H - 1
    for m in range(NT):
        sT_unit(h, m, pat)
    nc.vector.tensor_copy(aT[:, h // 2, :], pat)
    for m in range(NT):
        sn_unit(h, m)
        outproj_unit(wo_b, m)


def _build():
    nc = bacc.Bacc("TRN2", target_bir_lowering=False, debug=False)
    q_in = nc.dram_tensor("query", [T, D], F32, kind="ExternalInput")
    k_in = nc.dram_tensor("key", [T, D], F32, kind="ExternalInput")
    v_in = nc.dram_tensor("value", [T, D], F32, kind="ExternalInput")
    Wq = nc.dram_tensor("Wq", [D, H * KD], F32, kind="ExternalInput")
    Wk = nc.dram_tensor("Wk", [D, H * KD], F32, kind="ExternalInput")
    Wv = nc.dram_tensor("Wv", [D, H * KD], F32, kind="ExternalInput")
    Wo = nc.dram_tensor("Wo", [H * KD, D], F32, kind="ExternalInput")
    out_t = nc.dram_tensor("out", [T, D], F32, kind="ExternalOutput")
    attn_t = nc.dram_tensor("attn", [H, T, T], F32, kind="ExternalOutput")

    with tile.TileContext(nc) as tc:
        with ExitStack() as ctx:
            _body(ctx, nc, tc, q_in, k_in, v_in, Wq, Wk, Wv, Wo, out_t, attn_t)
    nc.compile()
    return nc


_NC = None


def _get_nc():
    global _NC
    if _NC is None:
        _NC = _build()
    return _NC


def run_sharded(in_maps, trace=False):
    nc = _get_nc()
    return bass_utils.run_bass_kernel_spmd(nc, in_maps, core_ids=list(range(B)),
                                           trace=trace)


def make_in_maps(query, key, value, Wq, Wk, Wv, Wo):
    query, key, value = (np.asarray(x, dtype=np.float32) for x in (query, key, value))
    Wq, Wk, Wv, Wo = (np.asarray(x, dtype=np.float32) for x in (Wq, Wk, Wv, Wo))
    return [
        {"query": np.ascontiguousarray(query[b]),
         "key": np.ascontiguousarray(key[b]),
         "value": np.ascontiguousarray(value[b]),
         "Wq": Wq, "Wk": Wk, "Wv": Wv, "Wo": Wo}
        for b in range(B)
    ]


def kernel(query, key, value, Wq, Wk, Wv, Wo):
    in_maps = make_in_maps(query, key, value, Wq, Wk, Wv, Wo)
    res = run_sharded(in_maps)
    out = np.stack([res.results[b]["out"] for b in range(B)])
    attn = np.stack([res.results[b]["attn"] for b in range(B)])
    return out, attn
